# revision 9
# baseline (speedup 1.0000x reference)
"""Multi-head causal attention (B=4, T=2048, D=1024, H=16, hd=64) on 8 TRN2 cores.

Sharding: tensor-parallel over heads — 2 heads per core, all batches. Each core:
  - QKV projections for its 128 output dims (fp32r matmuls, contraction over D)
  - scores computed TRANSPOSED (ST[k,q]) so no P transposes are needed
  - softmax without max-subtraction (scores bounded ~+-3); denominators come
    free from a ones-column appended to V; normalization via K=1 broadcast
    matmul + elementwise multiply, deferred one q-chunk off the critical path
  - partial output projection against its 128 columns of Wo
Emission is software-pipelined across batches: while the ACT engine grinds
through exp() for batch b's attention, the PE stream is fed filler work from
batch b+1's QKV projection and batch b-1's output projection.
Host: pre-transpose/round inputs to fp32r, sum the 8 partial outputs, add bias.
"""
import numpy as np

import concourse.bass as bass
import concourse.tile as tile
from concourse import bacc, mybir
from concourse.bass_utils import run_bass_kernel_spmd

F32 = mybir.dt.float32
F32R = mybir.dt.float32r
EXP = mybir.ActivationFunctionType.Exp

B, T, D = 4, 2048, 1024
NCORES = 8
TT = B * T        # 8192 tokens
CT = D // 128     # 8 contraction tiles
NQ = T // 512     # 4 q-chunks per batch
NK = T // 128     # 16 k-tiles per batch
LOOK = 4          # score->PV software-pipeline lookahead


def round_fp32r(a: np.ndarray) -> np.ndarray:
    """Round fp32 to fp32r (11 mantissa bits, low 12 bits zero), RNE."""
    u = np.ascontiguousarray(a, dtype=np.float32).view(np.uint32)
    r = (u + np.uint32(0x7FF) + ((u >> np.uint32(12)) & np.uint32(1))) & np.uint32(
        0xFFFFF000
    )
    return r.view(np.float32)


def build_nc():
    nc = bacc.Bacc(target_bir_lowering=False, num_devices=NCORES)
    xT_d = nc.declare_dram_parameter("xT", [D, TT], F32R, isOutput=False)
    wq_d = nc.declare_dram_parameter("wq", [128, D], F32R, isOutput=False)
    wk_d = nc.declare_dram_parameter("wk", [128, D], F32R, isOutput=False)
    wv_d = nc.declare_dram_parameter("wv", [128, D], F32R, isOutput=False)
    wo_d = nc.declare_dram_parameter("wo", [128, D], F32R, isOutput=False)
    tri_d = nc.declare_dram_parameter("tri", [128, 128], F32R, isOutput=False)
    ident_d = nc.declare_dram_parameter("ident", [128, 128], F32R, isOutput=False)
    onesr_d = nc.declare_dram_parameter("onesr", [1, 64], F32R, isOutput=False)
    ones32_d = nc.declare_dram_parameter("ones32", [128, 32], F32R, isOutput=False)
    out_d = nc.declare_dram_parameter("out", [TT, D], F32, isOutput=True)

    with tile.TileContext(nc) as tc:
        with tc.tile_pool(name="consts", bufs=1) as consts, \
             tc.tile_pool(name="xin", bufs=16) as xin, \
             tc.tile_pool(name="qkvp", bufs=2) as qkvp, \
             tc.tile_pool(name="attnp", bufs=2) as attnp, \
             tc.tile_pool(name="outp", bufs=3) as outp, \
             tc.tile_pool(name="ps_mm", bufs=4, space="PSUM") as ps_mm, \
             tc.tile_pool(name="ps_st", bufs=2, space="PSUM") as ps_st:

            wq_sb = consts.tile([128, D], F32R)
            wk_sb = consts.tile([128, D], F32R)
            wv_sb = consts.tile([128, D], F32R)
            wo_sb = consts.tile([128, D], F32R)
            tri_sb = consts.tile([128, 128], F32R)
            ident_sb = consts.tile([128, 128], F32R)
            onesr_sb = consts.tile([1, 64], F32R)
            ones32_sb = consts.tile([128, 32], F32R)
            nc.sync.dma_start(wq_sb[:], wq_d[:, :])
            nc.sync.dma_start(wk_sb[:], wk_d[:, :])
            nc.sync.dma_start(wv_sb[:], wv_d[:, :])
            nc.sync.dma_start(wo_sb[:], wo_d[:, :])
            nc.sync.dma_start(tri_sb[:], tri_d[:, :])
            nc.sync.dma_start(ident_sb[:], ident_d[:, :])
            nc.sync.dma_start(onesr_sb[:], onesr_d[:, :])
            nc.sync.dma_start(ones32_sb[:], ones32_d[:, :])

            state = {}  # per-batch tiles

            def alloc_batch(b):
                state[b] = {
                    "qt": qkvp.tile([128, T], F32R, name=f"qt{b}", tag="qt"),
                    "kt": qkvp.tile([128, T], F32R, name=f"kt{b}", tag="kt"),
                    "vt": qkvp.tile([128, T], F32R, name=f"vt{b}", tag="vt"),
                    "vaug": None, "ctxT": None, "ctx": {},
                }

            def qkv_chunk_ops(b, tcn):
                """Emitter closures for one 512-token QKV chunk of batch b."""
                s = state[b]
                ops = []
                xts = []

                def load_x(ct):
                    def f():
                        xt_t = xin.tile([128, 512], F32R,
                                        name=f"x{b}_{tcn}_{ct}", tag="xt")
                        nc.sync.dma_start(
                            xt_t[:],
                            xT_d[ct * 128:(ct + 1) * 128,
                                 b * T + tcn * 512: b * T + (tcn + 1) * 512])
                        xts.append(xt_t)
                    return f
                # issue all 8 DMAs up front (they run far ahead of use)
                def load_all():
                    for ct in range(CT):
                        load_x(ct)()
                ops.append(load_all)

                for wsb, dst in ((wq_sb, "qt"), (wk_sb, "kt"), (wv_sb, "vt")):
                    group = {}

                    def mk_mm(ct, wsb=wsb, group=group):
                        def f():
                            if ct == 0:
                                group["ps"] = ps_mm.tile(
                                    [128, 512], F32, name="mmps", tag="mm")
                            nc.tensor.matmul(
                                group["ps"][:], wsb[:, ct * 128:(ct + 1) * 128],
                                xts[ct][:], start=(ct == 0), stop=(ct == CT - 1))
                        return f

                    def mk_copy(dst=dst, group=group):
                        def f():
                            nc.vector.tensor_copy(
                                s[dst][:, tcn * 512:(tcn + 1) * 512],
                                group["ps"][:])
                        return f
                    for ct in range(CT):
                        ops.append(mk_mm(ct))
                    ops.append(mk_copy())
                return ops

            def vtrans_chunk_ops(b, c):
                """Transpose V chunk c (k-tiles 4c..4c+3) into vaug layout.

                vaug[:, k*130 + s*65 + 0:64] = V head s; col s*65+64 = 1.0
                """
                s = state[b]
                ops = []
                if s["vaug"] is None:
                    s["vaug"] = qkvp.tile([128, NK * 130], F32R,
                                          name=f"vaug{b}", tag="vaug")

                    def ones_f():
                        va4 = s["vaug"][:].rearrange(
                            "p (k s c) -> p k s c", k=NK, s=2)
                        nc.vector.tensor_copy(
                            va4[:, :, :, 64:65],
                            ones32_sb[:].rearrange(
                                "p (k s c) -> p k s c", k=NK, s=2))
                    ops.append(ones_f)
                for kj in range(4 * c, 4 * c + 4):
                    def mk(kj=kj):
                        def f():
                            vps = ps_mm.tile([128, 128], F32R,
                                             name="vps", tag="mm")
                            nc.tensor.transpose(
                                vps[:], s["vt"][:, kj * 128:(kj + 1) * 128],
                                ident_sb[:])
                            nc.vector.tensor_copy(
                                s["vaug"][:, kj * 130: kj * 130 + 130].rearrange(
                                    "p (s c) -> p s c", s=2)[:, :, 0:64],
                                vps[:].rearrange("p (s c) -> p s c", s=2))
                        return f
                    ops.append(mk(kj))
                return ops

            def proj_quarter_ops(b, qc):
                s = state[b]
                ops = []
                for tt_i in range(qc * 4, qc * 4 + 4):
                    def mk(tt_i=tt_i):
                        def f():
                            osb = outp.tile([128, D], F32, name="osb", tag="osb")
                            for oc in range(2):
                                op = ps_mm.tile([128, 512], F32,
                                                name="op", tag="mm")
                                nc.tensor.matmul(
                                    op[:],
                                    s["ctxT"][:, tt_i * 128:(tt_i + 1) * 128],
                                    wo_sb[:, oc * 512:(oc + 1) * 512],
                                    start=True, stop=True)
                                nc.vector.tensor_copy(
                                    osb[:, oc * 512:(oc + 1) * 512], op[:])
                            nc.sync.dma_start(
                                out_d[b * T + tt_i * 128:
                                      b * T + (tt_i + 1) * 128, :], osb[:])
                        return f
                    ops.append(mk(tt_i))
                return ops

            def kj_stream(b, qc, filler):
                """Scores+exp+mask+PV for both heads, filler ops interleaved."""
                s = state[b]
                if s["ctxT"] is None:
                    s["ctxT"] = qkvp.tile([128, T], F32R,
                                          name=f"ctxT{b}", tag="ctxT")
                kmax = (qc + 1) * 4
                qlo = qc * 512
                ctx0 = ps_mm.tile([65, 512], F32, name="ctx0", tag="mm")
                ctx1 = ps_mm.tile([65, 512], F32, name="ctx1", tag="mm")
                s["ctx"][qc] = (ctx0, ctx1)
                qt, kt, vaug = s["qt"], s["kt"], s["vaug"]
                pend = []
                nfill = len(filler)
                steps = kmax + LOOK
                done_f = 0
                for step in range(steps):
                    if step < kmax:
                        kj = step
                        off = max(0, kj * 128 - qlo)
                        ksl = slice(kj * 128, (kj + 1) * 128)
                        st = ps_st.tile([128, 1024], F32, name="st", tag="st")
                        nc.tensor.matmul(
                            st[:, off:512], kt[0:64, ksl],
                            qt[0:64, qlo + off: qlo + 512],
                            start=True, stop=True)
                        nc.tensor.matmul(
                            st[:, 512 + off:1024], kt[64:128, ksl],
                            qt[64:128, qlo + off: qlo + 512],
                            start=True, stop=True)
                        ptt = attnp.tile([128, 1024], F32R, name="pt",
                                         tag="pt", bufs=6)
                        nc.scalar.activation(
                            ptt[:, off:1024], st[:, off:1024], EXP, scale=0.125)
                        if kj * 128 >= qlo:  # diagonal: causal triangle mask
                            nc.gpsimd.tensor_mul(
                                ptt[:, off: off + 128],
                                ptt[:, off: off + 128], tri_sb[:])
                            nc.gpsimd.tensor_mul(
                                ptt[:, 512 + off: 512 + off + 128],
                                ptt[:, 512 + off: 512 + off + 128], tri_sb[:])
                        pend.append((kj, off, ptt))
                    if step >= LOOK and pend:
                        kj, off, ptt = pend.pop(0)
                        vb = kj * 130
                        nc.tensor.matmul(
                            ctx0[:, off:512], vaug[:, vb: vb + 65],
                            ptt[:, off:512],
                            start=(kj == 0), stop=(kj == kmax - 1),
                            skip_group_check=True)
                        nc.tensor.matmul(
                            ctx1[:, off:512], vaug[:, vb + 65: vb + 130],
                            ptt[:, 512 + off:1024],
                            start=(kj == 0), stop=(kj == kmax - 1),
                            skip_group_check=True)
                    # interleave filler to keep PE fed while ACT runs exp
                    want = nfill * (step + 1) // steps
                    while done_f < want:
                        filler[done_f]()
                        done_f += 1
                while pend:
                    kj, off, ptt = pend.pop(0)
                    vb = kj * 130
                    nc.tensor.matmul(
                        ctx0[:, off:512], vaug[:, vb: vb + 65], ptt[:, off:512],
                        start=(kj == 0), stop=(kj == kmax - 1),
                        skip_group_check=True)
                    nc.tensor.matmul(
                        ctx1[:, off:512], vaug[:, vb + 65: vb + 130],
                        ptt[:, 512 + off:1024],
                        start=(kj == 0), stop=(kj == kmax - 1),
                        skip_group_check=True)
                while done_f < nfill:
                    filler[done_f]()
                    done_f += 1

            def epilogue_pre(b, qc):
                """DVE-only part of the deferred normalization: 1/colsum."""
                s = state[b]
                recrs = []
                for h, ctx in zip((0, 1), s["ctx"][qc]):
                    deni = attnp.tile([1, 512], F32, name="deni", tag="deni")
                    nc.vector.tensor_copy(deni[:], ctx[64:65, :])
                    rec = attnp.tile([1, 512], F32, name="rec", tag="rec")
                    nc.vector.reciprocal_approx_fast(rec[:], deni[:])
                    recr = attnp.tile([1, 512], F32R, name="recr", tag="recr")
                    nc.vector.tensor_copy(recr[:], rec[:])
                    recrs.append(recr)
                return recrs

            def epilogue_ops(b, qc, recrs):
                """Broadcast + multiply; releases the ctx PSUM slots."""
                s = state[b]
                ctxs = s["ctx"].pop(qc)
                qlo = qc * 512
                ops = []
                for h in (0, 1):
                    def mk(h=h, ctx=ctxs[h], recr=recrs[h]):
                        def f():
                            bc = ps_st.tile([64, 512], F32, name="bc", tag="st")
                            nc.tensor.matmul(bc[:], onesr_sb[:], recr[:],
                                             start=True, stop=True)
                            bcs = attnp.tile([64, 512], F32, name="bcs",
                                             tag="bcs")
                            nc.vector.tensor_copy(bcs[:], bc[:])
                            nc.vector.tensor_mul(
                                s["ctxT"][h * 64:(h + 1) * 64, qlo: qlo + 512],
                                ctx[0:64, :], bcs[:])
                        return f
                    ops.append(mk(h))
                return ops

            # ---- prologue: batch 0 QKV + V-transpose, emitted densely ----
            alloc_batch(0)
            for tcn in range(NQ):
                for f in qkv_chunk_ops(0, tcn):
                    f()
            for c in range(NK // 4):
                for f in vtrans_chunk_ops(0, c):
                    f()

            # ---- pipelined main loop ----
            prev = None  # (b, qc) awaiting epilogue
            for b in range(B):
                if b + 1 < B:
                    alloc_batch(b + 1)
                for qc in range(NQ):
                    filler = []
                    if prev is not None:
                        recrs = epilogue_pre(*prev)
                        filler += epilogue_ops(*prev, recrs)
                    if b + 1 < B:
                        filler += qkv_chunk_ops(b + 1, qc)
                        if qc >= 1:
                            filler += vtrans_chunk_ops(b + 1, qc - 1)
                    if b >= 1 and qc == 0:
                        filler += vtrans_chunk_ops(b, 3)
                    if b >= 1:
                        filler += proj_quarter_ops(b - 1, qc)
                    if b == B - 1 and qc >= 1:
                        filler += proj_quarter_ops(b, qc - 1)
                    kj_stream(b, qc, filler)
                    prev = (b, qc)
            recrs = epilogue_pre(*prev)
            for f in epilogue_ops(*prev, recrs):
                f()
            for f in proj_quarter_ops(B - 1, 3):
                f()

    nc.compile()
    return nc


def make_in_maps(x, Wq, Wk, Wv, Wo):
    xT = round_fp32r(np.ascontiguousarray(x.reshape(TT, D).T))
    tri = round_fp32r(np.triu(np.ones((128, 128), np.float32)))  # keep k<=q
    ident = round_fp32r(np.eye(128, dtype=np.float32))
    onesr = np.ones((1, 64), np.float32)
    ones32 = np.ones((128, 32), np.float32)
    in_maps = []
    for c in range(NCORES):
        dsl = slice(c * 128, (c + 1) * 128)
        wq = np.concatenate(
            [Wq[dsl, ct * 128:(ct + 1) * 128].T for ct in range(CT)], axis=1)
        wk = np.concatenate(
            [Wk[dsl, ct * 128:(ct + 1) * 128].T for ct in range(CT)], axis=1)
        wv = np.concatenate(
            [Wv[dsl, ct * 128:(ct + 1) * 128].T for ct in range(CT)], axis=1)
        wo = Wo[:, dsl].T
        in_maps.append({
            "xT": xT,
            "wq": round_fp32r(np.ascontiguousarray(wq)),
            "wk": round_fp32r(np.ascontiguousarray(wk)),
            "wv": round_fp32r(np.ascontiguousarray(wv)),
            "wo": round_fp32r(np.ascontiguousarray(wo)),
            "tri": tri, "ident": ident, "onesr": onesr, "ones32": ones32,
        })
    return in_maps


_NC_CACHE = None


def kernel_run(x, Wq, Wk, Wv, Wo, bo, trace=False, trace_cores=None):
    global _NC_CACHE
    if _NC_CACHE is None:
        _NC_CACHE = build_nc()
    nc = _NC_CACHE
    in_maps = make_in_maps(np.asarray(x), np.asarray(Wq), np.asarray(Wk),
                           np.asarray(Wv), np.asarray(Wo))
    res = run_bass_kernel_spmd(nc, in_maps, core_ids=list(range(NCORES)),
                               trace=trace, trace_cores=trace_cores)
    acc = res.results[0]["out"].astype(np.float64)
    for c in range(1, NCORES):
        acc += res.results[c]["out"]
    outv = (acc + np.asarray(bo, dtype=np.float64)).astype(np.float32)
    return outv.reshape(B, T, D), res


def kernel(x, Wq, Wk, Wv, Wo, bo):
    out, _ = kernel_run(x, Wq, Wk, Wv, Wo, bo)
    return out


# revision 11
# speedup vs baseline: 1.0479x; 1.0479x over previous
"""Multi-head causal attention (B=4, T=2048, D=1024, H=16, hd=64) on 8 TRN2 cores.

Sharding: tensor-parallel over heads — 2 heads per core, all batches. Each core:
  - QKV projections for its 128 output dims (fp32r matmuls, contraction over D)
  - scores computed TRANSPOSED (ST[k,q]) so no P transposes are needed
  - softmax without max-subtraction (scores bounded ~+-3); denominators come
    free from a ones-column appended to V; normalization via K=1 broadcast
    matmul + elementwise multiply, deferred one q-chunk off the critical path
  - partial output projection against its 128 columns of Wo
Emission is software-pipelined across batches: while the ACT engine grinds
through exp() for batch b's attention, the PE stream is fed filler work from
batch b+1's QKV projection and batch b-1's output projection.
Host: pre-transpose/round inputs to fp32r, sum the 8 partial outputs, add bias.
"""
import numpy as np

import concourse.bass as bass
import concourse.tile as tile
from concourse import bacc, mybir
from concourse.bass_utils import run_bass_kernel_spmd

F32 = mybir.dt.float32
F32R = mybir.dt.float32r
EXP = mybir.ActivationFunctionType.Exp

B, T, D = 4, 2048, 1024
NCORES = 8
TT = B * T        # 8192 tokens
CT = D // 128     # 8 contraction tiles
NQ = T // 512     # 4 q-chunks per batch
NK = T // 128     # 16 k-tiles per batch
LOOK = 4          # score->PV software-pipeline lookahead


def round_fp32r(a: np.ndarray) -> np.ndarray:
    """Round fp32 to fp32r (11 mantissa bits, low 12 bits zero), RNE."""
    u = np.ascontiguousarray(a, dtype=np.float32).view(np.uint32)
    r = (u + np.uint32(0x7FF) + ((u >> np.uint32(12)) & np.uint32(1))) & np.uint32(
        0xFFFFF000
    )
    return r.view(np.float32)


def build_nc():
    nc = bacc.Bacc(target_bir_lowering=False, num_devices=NCORES)
    xT_d = nc.declare_dram_parameter("xT", [D, TT], F32R, isOutput=False)
    wq_d = nc.declare_dram_parameter("wq", [128, D], F32R, isOutput=False)
    wk_d = nc.declare_dram_parameter("wk", [128, D], F32R, isOutput=False)
    wv_d = nc.declare_dram_parameter("wv", [128, D], F32R, isOutput=False)
    wo_d = nc.declare_dram_parameter("wo", [128, D], F32R, isOutput=False)
    tri_d = nc.declare_dram_parameter("tri", [128, 128], F32R, isOutput=False)
    ident_d = nc.declare_dram_parameter("ident", [128, 128], F32R, isOutput=False)
    onesr_d = nc.declare_dram_parameter("onesr", [1, 64], F32R, isOutput=False)
    ones32_d = nc.declare_dram_parameter("ones32", [128, 32], F32R, isOutput=False)
    out_d = nc.declare_dram_parameter("out", [TT, D], F32, isOutput=True)

    with tile.TileContext(nc) as tc:
        with tc.tile_pool(name="consts", bufs=1) as consts, \
             tc.tile_pool(name="xin", bufs=16) as xin, \
             tc.tile_pool(name="qkvp", bufs=2) as qkvp, \
             tc.tile_pool(name="attnp", bufs=2) as attnp, \
             tc.tile_pool(name="outp", bufs=3) as outp, \
             tc.tile_pool(name="ps_mm", bufs=4, space="PSUM") as ps_mm, \
             tc.tile_pool(name="ps_st", bufs=2, space="PSUM") as ps_st:

            wq_sb = consts.tile([128, D], F32R)
            wk_sb = consts.tile([128, D], F32R)
            wv_sb = consts.tile([128, D], F32R)
            wo_sb = consts.tile([128, D], F32R)
            tri_sb = consts.tile([128, 128], F32R)
            ident_sb = consts.tile([128, 128], F32R)
            onesr_sb = consts.tile([1, 64], F32R)
            ones32_sb = consts.tile([128, 32], F32R)
            nc.sync.dma_start(wq_sb[:], wq_d[:, :])
            nc.sync.dma_start(wk_sb[:], wk_d[:, :])
            nc.sync.dma_start(wv_sb[:], wv_d[:, :])
            nc.sync.dma_start(wo_sb[:], wo_d[:, :])
            nc.sync.dma_start(tri_sb[:], tri_d[:, :])
            nc.sync.dma_start(ident_sb[:], ident_d[:, :])
            nc.sync.dma_start(onesr_sb[:], onesr_d[:, :])
            nc.sync.dma_start(ones32_sb[:], ones32_d[:, :])

            state = {}  # per-batch tiles

            def alloc_batch(b):
                state[b] = {
                    "qt": qkvp.tile([128, T], F32R, name=f"qt{b}", tag="qt"),
                    "kt": qkvp.tile([128, T], F32R, name=f"kt{b}", tag="kt"),
                    "vt": qkvp.tile([128, T], F32R, name=f"vt{b}", tag="vt"),
                    "vaug": None, "ctxT": None, "ctx": {},
                }

            def qkv_chunk_ops(b, tcn):
                """Emitter closures for one 512-token QKV chunk of batch b."""
                s = state[b]
                xts = []

                def load_x(ct):
                    def f():
                        xt_t = xin.tile([128, 512], F32R,
                                        name=f"x{b}_{tcn}_{ct}", tag="xt")
                        nc.sync.dma_start(
                            xt_t[:],
                            xT_d[ct * 128:(ct + 1) * 128,
                                 b * T + tcn * 512: b * T + (tcn + 1) * 512])
                        xts.append(xt_t)
                    return f
                # issue all 8 DMAs up front (they run far ahead of use)
                def load_all():
                    for ct in range(CT):
                        load_x(ct)()

                first = [load_all]
                chunks = [first]
                for wsb, dst in ((wq_sb, "qt"), (wk_sb, "kt"), (wv_sb, "vt")):
                    def mk_group(wsb=wsb, dst=dst):
                        def f():
                            ps = ps_mm.tile([128, 512], F32, name="mmps",
                                            tag="mm")
                            for ct in range(CT):
                                nc.tensor.matmul(
                                    ps[:], wsb[:, ct * 128:(ct + 1) * 128],
                                    xts[ct][:], start=(ct == 0),
                                    stop=(ct == CT - 1))
                            nc.vector.tensor_copy(
                                s[dst][:, tcn * 512:(tcn + 1) * 512], ps[:])
                        return f
                    chunks.append([mk_group()])
                return chunks

            def vtrans_chunk_ops(b, c):
                """Transpose V chunk c (k-tiles 4c..4c+3) into vaug layout.

                vaug[:, k*130 + s*65 + 0:64] = V head s; col s*65+64 = 1.0
                """
                s = state[b]
                chunks = []
                ops = []
                if s["vaug"] is None:
                    s["vaug"] = qkvp.tile([128, NK * 130], F32R,
                                          name=f"vaug{b}", tag="vaug")

                    def ones_f():
                        va4 = s["vaug"][:].rearrange(
                            "p (k s c) -> p k s c", k=NK, s=2)
                        nc.vector.tensor_copy(
                            va4[:, :, :, 64:65],
                            ones32_sb[:].rearrange(
                                "p (k s c) -> p k s c", k=NK, s=2))
                    ops.append(ones_f)
                for kj in range(4 * c, 4 * c + 4):
                    def mk(kj=kj):
                        def f():
                            vps = ps_mm.tile([128, 128], F32R,
                                             name="vps", tag="mm")
                            nc.tensor.transpose(
                                vps[:], s["vt"][:, kj * 128:(kj + 1) * 128],
                                ident_sb[:])
                            nc.vector.tensor_copy(
                                s["vaug"][:, kj * 130: kj * 130 + 130].rearrange(
                                    "p (s c) -> p s c", s=2)[:, :, 0:64],
                                vps[:].rearrange("p (s c) -> p s c", s=2))
                        return f
                    ops.append(mk(kj))
                    if len(ops) == 2:
                        chunks.append(ops)
                        ops = []
                if ops:
                    chunks.append(ops)
                return chunks

            def proj_quarter_ops(b, qc):
                s = state[b]
                chunks = []
                ops = []
                for tt_i in range(qc * 4, qc * 4 + 4):
                    def mk(tt_i=tt_i):
                        def f():
                            osb = outp.tile([128, D], F32, name="osb", tag="osb")
                            for oc in range(2):
                                op = ps_mm.tile([128, 512], F32,
                                                name="op", tag="mm")
                                nc.tensor.matmul(
                                    op[:],
                                    s["ctxT"][:, tt_i * 128:(tt_i + 1) * 128],
                                    wo_sb[:, oc * 512:(oc + 1) * 512],
                                    start=True, stop=True)
                                nc.vector.tensor_copy(
                                    osb[:, oc * 512:(oc + 1) * 512], op[:])
                            nc.sync.dma_start(
                                out_d[b * T + tt_i * 128:
                                      b * T + (tt_i + 1) * 128, :], osb[:])
                        return f
                    ops.append(mk(tt_i))
                    if len(ops) == 2:
                        chunks.append(ops)
                        ops = []
                if ops:
                    chunks.append(ops)
                return chunks

            def kj_stream(b, qc, filler):
                """Scores+exp+mask+PV for both heads, filler ops interleaved."""
                s = state[b]
                if s["ctxT"] is None:
                    s["ctxT"] = qkvp.tile([128, T], F32R,
                                          name=f"ctxT{b}", tag="ctxT")
                kmax = (qc + 1) * 4
                qlo = qc * 512
                ctx0 = ps_mm.tile([65, 512], F32, name="ctx0", tag="mm")
                ctx1 = ps_mm.tile([65, 512], F32, name="ctx1", tag="mm")
                s["ctx"][qc] = (ctx0, ctx1)
                qt, kt, vaug = s["qt"], s["kt"], s["vaug"]
                pend = []
                nfill = len(filler)
                steps = kmax + LOOK
                done_f = 0

                def pop_filler(upto):
                    nonlocal done_f
                    while done_f < upto:
                        for f in filler[done_f]:
                            f()
                        done_f += 1
                for step in range(steps):
                    if step < kmax:
                        kj = step
                        off = max(0, kj * 128 - qlo)
                        ksl = slice(kj * 128, (kj + 1) * 128)
                        st = ps_st.tile([128, 1024], F32, name="st", tag="st")
                        nc.tensor.matmul(
                            st[:, off:512], kt[0:64, ksl],
                            qt[0:64, qlo + off: qlo + 512],
                            start=True, stop=True)
                        nc.tensor.matmul(
                            st[:, 512 + off:1024], kt[64:128, ksl],
                            qt[64:128, qlo + off: qlo + 512],
                            start=True, stop=True)
                        ptt = attnp.tile([128, 1024], F32R, name="pt",
                                         tag="pt", bufs=6)
                        nc.scalar.activation(
                            ptt[:, off:1024], st[:, off:1024], EXP, scale=0.125)
                        if kj * 128 >= qlo:  # diagonal: causal triangle mask
                            nc.gpsimd.tensor_mul(
                                ptt[:, off: off + 128],
                                ptt[:, off: off + 128], tri_sb[:])
                            nc.gpsimd.tensor_mul(
                                ptt[:, 512 + off: 512 + off + 128],
                                ptt[:, 512 + off: 512 + off + 128], tri_sb[:])
                        pend.append((kj, off, ptt))
                    if step >= LOOK and pend:
                        kj, off, ptt = pend.pop(0)
                        vb = kj * 130
                        nc.tensor.matmul(
                            ctx0[:, off:512], vaug[:, vb: vb + 65],
                            ptt[:, off:512],
                            start=(kj == 0), stop=(kj == kmax - 1),
                            skip_group_check=True)
                        nc.tensor.matmul(
                            ctx1[:, off:512], vaug[:, vb + 65: vb + 130],
                            ptt[:, 512 + off:1024],
                            start=(kj == 0), stop=(kj == kmax - 1),
                            skip_group_check=True)
                    # burst filler (2 chunks ~3.4us dense PE) to re-warm HAM
                    want = nfill * (step + 1) // steps
                    if want - done_f >= 2 or step >= steps - 1:
                        pop_filler(want)
                while pend:
                    kj, off, ptt = pend.pop(0)
                    vb = kj * 130
                    nc.tensor.matmul(
                        ctx0[:, off:512], vaug[:, vb: vb + 65], ptt[:, off:512],
                        start=(kj == 0), stop=(kj == kmax - 1),
                        skip_group_check=True)
                    nc.tensor.matmul(
                        ctx1[:, off:512], vaug[:, vb + 65: vb + 130],
                        ptt[:, 512 + off:1024],
                        start=(kj == 0), stop=(kj == kmax - 1),
                        skip_group_check=True)
                pop_filler(nfill)

            def epilogue_pre(b, qc):
                """DVE-only part of the deferred normalization: 1/colsum."""
                s = state[b]
                recrs = []
                for h, ctx in zip((0, 1), s["ctx"][qc]):
                    deni = attnp.tile([1, 512], F32, name="deni", tag="deni")
                    nc.vector.tensor_copy(deni[:], ctx[64:65, :])
                    rec = attnp.tile([1, 512], F32, name="rec", tag="rec")
                    nc.vector.reciprocal_approx_fast(rec[:], deni[:])
                    recr = attnp.tile([1, 512], F32R, name="recr", tag="recr")
                    nc.vector.tensor_copy(recr[:], rec[:])
                    recrs.append(recr)
                return recrs

            def epilogue_ops(b, qc, recrs):
                """Broadcast + multiply; releases the ctx PSUM slots."""
                s = state[b]
                ctxs = s["ctx"].pop(qc)
                qlo = qc * 512
                ops = []
                for h in (0, 1):
                    def mk(h=h, ctx=ctxs[h], recr=recrs[h]):
                        def f():
                            bc = ps_st.tile([64, 512], F32, name="bc", tag="st")
                            nc.tensor.matmul(bc[:], onesr_sb[:], recr[:],
                                             start=True, stop=True)
                            bcs = attnp.tile([64, 512], F32, name="bcs",
                                             tag="bcs")
                            nc.vector.tensor_copy(bcs[:], bc[:])
                            nc.vector.tensor_mul(
                                s["ctxT"][h * 64:(h + 1) * 64, qlo: qlo + 512],
                                ctx[0:64, :], bcs[:])
                        return f
                    ops.append(mk(h))
                return [ops]

            # ---- prologue: batch 0 QKV + V-transpose, emitted densely ----
            alloc_batch(0)
            for tcn in range(NQ):
                for ch in qkv_chunk_ops(0, tcn):
                    for f in ch:
                        f()
            for c in range(NK // 4):
                for ch in vtrans_chunk_ops(0, c):
                    for f in ch:
                        f()

            # ---- pipelined main loop ----
            prev = None  # (b, qc) awaiting epilogue
            for b in range(B):
                if b + 1 < B:
                    alloc_batch(b + 1)
                for qc in range(NQ):
                    filler = []
                    if prev is not None:
                        recrs = epilogue_pre(*prev)
                        filler += epilogue_ops(*prev, recrs)
                    if b + 1 < B:
                        filler += qkv_chunk_ops(b + 1, qc)
                        if qc >= 1:
                            filler += vtrans_chunk_ops(b + 1, qc - 1)
                    if b >= 1 and qc == 0:
                        filler += vtrans_chunk_ops(b, 3)
                    if b >= 1:
                        filler += proj_quarter_ops(b - 1, qc)
                    if b == B - 1 and qc >= 1:
                        filler += proj_quarter_ops(b, qc - 1)
                    kj_stream(b, qc, filler)
                    prev = (b, qc)
            recrs = epilogue_pre(*prev)
            for ch in epilogue_ops(*prev, recrs):
                for f in ch:
                    f()
            for ch in proj_quarter_ops(B - 1, 3):
                for f in ch:
                    f()

    nc.compile()
    return nc


def make_in_maps(x, Wq, Wk, Wv, Wo):
    xT = round_fp32r(np.ascontiguousarray(x.reshape(TT, D).T))
    tri = round_fp32r(np.triu(np.ones((128, 128), np.float32)))  # keep k<=q
    ident = round_fp32r(np.eye(128, dtype=np.float32))
    onesr = np.ones((1, 64), np.float32)
    ones32 = np.ones((128, 32), np.float32)
    in_maps = []
    for c in range(NCORES):
        dsl = slice(c * 128, (c + 1) * 128)
        wq = np.concatenate(
            [Wq[dsl, ct * 128:(ct + 1) * 128].T for ct in range(CT)], axis=1)
        wk = np.concatenate(
            [Wk[dsl, ct * 128:(ct + 1) * 128].T for ct in range(CT)], axis=1)
        wv = np.concatenate(
            [Wv[dsl, ct * 128:(ct + 1) * 128].T for ct in range(CT)], axis=1)
        wo = Wo[:, dsl].T
        in_maps.append({
            "xT": xT,
            "wq": round_fp32r(np.ascontiguousarray(wq)),
            "wk": round_fp32r(np.ascontiguousarray(wk)),
            "wv": round_fp32r(np.ascontiguousarray(wv)),
            "wo": round_fp32r(np.ascontiguousarray(wo)),
            "tri": tri, "ident": ident, "onesr": onesr, "ones32": ones32,
        })
    return in_maps


_NC_CACHE = None


def kernel_run(x, Wq, Wk, Wv, Wo, bo, trace=False, trace_cores=None):
    global _NC_CACHE
    if _NC_CACHE is None:
        _NC_CACHE = build_nc()
    nc = _NC_CACHE
    in_maps = make_in_maps(np.asarray(x), np.asarray(Wq), np.asarray(Wk),
                           np.asarray(Wv), np.asarray(Wo))
    res = run_bass_kernel_spmd(nc, in_maps, core_ids=list(range(NCORES)),
                               trace=trace, trace_cores=trace_cores)
    acc = res.results[0]["out"].astype(np.float64)
    for c in range(1, NCORES):
        acc += res.results[c]["out"]
    outv = (acc + np.asarray(bo, dtype=np.float64)).astype(np.float32)
    return outv.reshape(B, T, D), res


def kernel(x, Wq, Wk, Wv, Wo, bo):
    out, _ = kernel_run(x, Wq, Wk, Wv, Wo, bo)
    return out


# revision 12
# speedup vs baseline: 1.0777x; 1.0284x over previous
"""Multi-head causal attention (B=4, T=2048, D=1024, H=16, hd=64) on 8 TRN2 cores.

Sharding: tensor-parallel over heads — 2 heads per core, all batches. Each core:
  - QKV projections for its 128 output dims (fp32r matmuls, contraction over D)
  - scores computed TRANSPOSED (ST[k,q]) so no P transposes are needed
  - softmax without max-subtraction (scores bounded ~+-3); denominators come
    free from a ones-column appended to V; normalization via K=1 broadcast
    matmul + elementwise multiply, deferred one q-chunk off the critical path
  - partial output projection against its 128 columns of Wo
Emission is software-pipelined across batches: while the ACT engine grinds
through exp() for batch b's attention, the PE stream is fed filler work from
batch b+1's QKV projection and batch b-1's output projection.
Host: pre-transpose/round inputs to fp32r, sum the 8 partial outputs, add bias.
"""
import numpy as np

import concourse.bass as bass
import concourse.tile as tile
from concourse import bacc, mybir
from concourse.bass_utils import run_bass_kernel_spmd

F32 = mybir.dt.float32
F32R = mybir.dt.float32r
EXP = mybir.ActivationFunctionType.Exp

B, T, D = 4, 2048, 1024
NCORES = 8
TT = B * T        # 8192 tokens
CT = D // 128     # 8 contraction tiles
NQ = T // 512     # 4 q-chunks per batch
NK = T // 128     # 16 k-tiles per batch
LOOK = 4          # score->PV software-pipeline lookahead


def round_fp32r(a: np.ndarray) -> np.ndarray:
    """Round fp32 to fp32r (11 mantissa bits, low 12 bits zero), RNE."""
    u = np.ascontiguousarray(a, dtype=np.float32).view(np.uint32)
    r = (u + np.uint32(0x7FF) + ((u >> np.uint32(12)) & np.uint32(1))) & np.uint32(
        0xFFFFF000
    )
    return r.view(np.float32)


def build_nc():
    nc = bacc.Bacc(target_bir_lowering=False, num_devices=NCORES)
    xT_d = nc.declare_dram_parameter("xT", [D, TT], F32R, isOutput=False)
    wq_d = nc.declare_dram_parameter("wq", [128, D], F32R, isOutput=False)
    wk_d = nc.declare_dram_parameter("wk", [128, D], F32R, isOutput=False)
    wv_d = nc.declare_dram_parameter("wv", [128, D], F32R, isOutput=False)
    wo_d = nc.declare_dram_parameter("wo", [128, D], F32R, isOutput=False)
    tri_d = nc.declare_dram_parameter("tri", [128, 128], F32R, isOutput=False)
    ident_d = nc.declare_dram_parameter("ident", [128, 128], F32R, isOutput=False)
    onesr_d = nc.declare_dram_parameter("onesr", [1, 64], F32R, isOutput=False)
    ones32_d = nc.declare_dram_parameter("ones32", [128, 32], F32R, isOutput=False)
    out_d = nc.declare_dram_parameter("out", [TT, D], F32, isOutput=True)

    with tile.TileContext(nc) as tc:
        with tc.tile_pool(name="consts", bufs=1) as consts, \
             tc.tile_pool(name="xin", bufs=16) as xin, \
             tc.tile_pool(name="qkvp", bufs=2) as qkvp, \
             tc.tile_pool(name="attnp", bufs=2) as attnp, \
             tc.tile_pool(name="outp", bufs=3) as outp, \
             tc.tile_pool(name="ps_mm", bufs=4, space="PSUM") as ps_mm, \
             tc.tile_pool(name="ps_st", bufs=2, space="PSUM") as ps_st:

            wq_sb = consts.tile([128, D], F32R)
            wk_sb = consts.tile([128, D], F32R)
            wv_sb = consts.tile([128, D], F32R)
            wo_sb = consts.tile([128, D], F32R)
            tri_sb = consts.tile([128, 128], F32R)
            ident_sb = consts.tile([128, 128], F32R)
            onesr_sb = consts.tile([1, 64], F32R)
            ones32_sb = consts.tile([128, 32], F32R)
            nc.sync.dma_start(wq_sb[:], wq_d[:, :])
            nc.sync.dma_start(wk_sb[:], wk_d[:, :])
            nc.sync.dma_start(wv_sb[:], wv_d[:, :])
            nc.sync.dma_start(wo_sb[:], wo_d[:, :])
            nc.sync.dma_start(tri_sb[:], tri_d[:, :])
            nc.sync.dma_start(ident_sb[:], ident_d[:, :])
            nc.sync.dma_start(onesr_sb[:], onesr_d[:, :])
            nc.sync.dma_start(ones32_sb[:], ones32_d[:, :])

            state = {}  # per-batch tiles

            def alloc_batch(b):
                state[b] = {
                    "qt": qkvp.tile([128, T], F32R, name=f"qt{b}", tag="qt"),
                    "kt": qkvp.tile([128, T], F32R, name=f"kt{b}", tag="kt"),
                    "vt": qkvp.tile([128, T], F32R, name=f"vt{b}", tag="vt"),
                    "vaug": None, "ctxT": None, "ctx": {},
                }

            def qkv_chunk_ops(b, tcn):
                """Emitter closures for one 512-token QKV chunk of batch b."""
                s = state[b]
                xts = []

                def load_x(ct):
                    def f():
                        xt_t = xin.tile([128, 512], F32R,
                                        name=f"x{b}_{tcn}_{ct}", tag="xt")
                        nc.sync.dma_start(
                            xt_t[:],
                            xT_d[ct * 128:(ct + 1) * 128,
                                 b * T + tcn * 512: b * T + (tcn + 1) * 512])
                        xts.append(xt_t)
                    return f
                # issue all 8 DMAs up front (they run far ahead of use)
                def load_all():
                    for ct in range(CT):
                        load_x(ct)()

                first = [load_all]
                chunks = [first]
                for wsb, dst in ((wq_sb, "qt"), (wk_sb, "kt"), (wv_sb, "vt")):
                    def mk_group(wsb=wsb, dst=dst):
                        def f():
                            ps = ps_mm.tile([128, 512], F32, name="mmps",
                                            tag="mm")
                            for ct in range(CT):
                                nc.tensor.matmul(
                                    ps[:], wsb[:, ct * 128:(ct + 1) * 128],
                                    xts[ct][:], start=(ct == 0),
                                    stop=(ct == CT - 1))
                            nc.vector.tensor_copy(
                                s[dst][:, tcn * 512:(tcn + 1) * 512], ps[:])
                        return f
                    chunks.append([mk_group()])
                return chunks

            def vtrans_chunk_ops(b, c):
                """Transpose V chunk c (k-tiles 4c..4c+3) into vaug layout.

                vaug[:, k*130 + s*65 + 0:64] = V head s; col s*65+64 = 1.0
                """
                s = state[b]
                chunks = []
                ops = []
                if s["vaug"] is None:
                    s["vaug"] = qkvp.tile([128, NK * 130], F32R,
                                          name=f"vaug{b}", tag="vaug")

                    def ones_f():
                        va4 = s["vaug"][:].rearrange(
                            "p (k s c) -> p k s c", k=NK, s=2)
                        nc.vector.tensor_copy(
                            va4[:, :, :, 64:65],
                            ones32_sb[:].rearrange(
                                "p (k s c) -> p k s c", k=NK, s=2))
                    ops.append(ones_f)
                for kj in range(4 * c, 4 * c + 4):
                    def mk(kj=kj):
                        def f():
                            vps = ps_mm.tile([128, 128], F32R,
                                             name="vps", tag="mm")
                            nc.tensor.transpose(
                                vps[:], s["vt"][:, kj * 128:(kj + 1) * 128],
                                ident_sb[:])
                            nc.vector.tensor_copy(
                                s["vaug"][:, kj * 130: kj * 130 + 130].rearrange(
                                    "p (s c) -> p s c", s=2)[:, :, 0:64],
                                vps[:].rearrange("p (s c) -> p s c", s=2))
                        return f
                    ops.append(mk(kj))
                    if len(ops) == 2:
                        chunks.append(ops)
                        ops = []
                if ops:
                    chunks.append(ops)
                return chunks

            def proj_quarter_ops(b, qc):
                s = state[b]
                chunks = []
                ops = []
                for tt_i in range(qc * 4, qc * 4 + 4):
                    def mk(tt_i=tt_i):
                        def f():
                            osb = outp.tile([128, D], F32, name="osb", tag="osb")
                            for oc in range(2):
                                op = ps_mm.tile([128, 512], F32,
                                                name="op", tag="mm")
                                nc.tensor.matmul(
                                    op[:],
                                    s["ctxT"][:, tt_i * 128:(tt_i + 1) * 128],
                                    wo_sb[:, oc * 512:(oc + 1) * 512],
                                    start=True, stop=True)
                                nc.vector.tensor_copy(
                                    osb[:, oc * 512:(oc + 1) * 512], op[:])
                            nc.sync.dma_start(
                                out_d[b * T + tt_i * 128:
                                      b * T + (tt_i + 1) * 128, :], osb[:])
                        return f
                    ops.append(mk(tt_i))
                    if len(ops) == 2:
                        chunks.append(ops)
                        ops = []
                if ops:
                    chunks.append(ops)
                return chunks

            def kj_stream(b, qc, filler):
                """Scores+exp+mask+PV for both heads, filler ops interleaved."""
                s = state[b]
                if s["ctxT"] is None:
                    s["ctxT"] = qkvp.tile([128, T], F32R,
                                          name=f"ctxT{b}", tag="ctxT")
                kmax = (qc + 1) * 4
                qlo = qc * 512
                ctx0 = ps_mm.tile([65, 512], F32, name="ctx0", tag="mm")
                ctx1 = ps_mm.tile([65, 512], F32, name="ctx1", tag="mm")
                s["ctx"][qc] = (ctx0, ctx1)
                qt, kt, vaug = s["qt"], s["kt"], s["vaug"]
                pend = []
                nfill = len(filler)
                steps = kmax + LOOK
                done_f = 0

                def pop_filler(upto):
                    nonlocal done_f
                    while done_f < upto:
                        for f in filler[done_f]:
                            f()
                        done_f += 1
                for step in range(steps):
                    if step < kmax:
                        kj = step
                        off = max(0, kj * 128 - qlo)
                        ksl = slice(kj * 128, (kj + 1) * 128)
                        st = ps_st.tile([128, 1024], F32, name="st", tag="st")
                        nc.tensor.matmul(
                            st[:, off:512], kt[0:64, ksl],
                            qt[0:64, qlo + off: qlo + 512],
                            start=True, stop=True)
                        nc.tensor.matmul(
                            st[:, 512 + off:1024], kt[64:128, ksl],
                            qt[64:128, qlo + off: qlo + 512],
                            start=True, stop=True)
                        ptt = attnp.tile([128, 1024], F32R, name="pt",
                                         tag="pt", bufs=6)
                        nc.scalar.activation(
                            ptt[:, off:1024], st[:, off:1024], EXP, scale=0.125)
                        if kj * 128 >= qlo:  # diagonal: causal triangle mask
                            nc.gpsimd.tensor_mul(
                                ptt[:, off: off + 128],
                                ptt[:, off: off + 128], tri_sb[:])
                            nc.gpsimd.tensor_mul(
                                ptt[:, 512 + off: 512 + off + 128],
                                ptt[:, 512 + off: 512 + off + 128], tri_sb[:])
                        pend.append((kj, off, ptt))
                    if step >= LOOK and pend:
                        kj, off, ptt = pend.pop(0)
                        vb = kj * 130
                        nc.tensor.matmul(
                            ctx0[:, off:512], vaug[:, vb: vb + 65],
                            ptt[:, off:512],
                            start=(kj == 0), stop=(kj == kmax - 1),
                            skip_group_check=True)
                        nc.tensor.matmul(
                            ctx1[:, off:512], vaug[:, vb + 65: vb + 130],
                            ptt[:, 512 + off:1024],
                            start=(kj == 0), stop=(kj == kmax - 1),
                            skip_group_check=True)
                    # burst filler (2 chunks ~3.4us dense PE) to re-warm HAM
                    want = nfill * (step + 1) // steps
                    if want - done_f >= 3 or step >= steps - 1:
                        pop_filler(want)
                while pend:
                    kj, off, ptt = pend.pop(0)
                    vb = kj * 130
                    nc.tensor.matmul(
                        ctx0[:, off:512], vaug[:, vb: vb + 65], ptt[:, off:512],
                        start=(kj == 0), stop=(kj == kmax - 1),
                        skip_group_check=True)
                    nc.tensor.matmul(
                        ctx1[:, off:512], vaug[:, vb + 65: vb + 130],
                        ptt[:, 512 + off:1024],
                        start=(kj == 0), stop=(kj == kmax - 1),
                        skip_group_check=True)
                pop_filler(nfill)

            def epilogue_pre(b, qc):
                """DVE-only part of the deferred normalization: 1/colsum."""
                s = state[b]
                recrs = []
                for h, ctx in zip((0, 1), s["ctx"][qc]):
                    deni = attnp.tile([1, 512], F32, name="deni", tag="deni")
                    nc.vector.tensor_copy(deni[:], ctx[64:65, :])
                    rec = attnp.tile([1, 512], F32, name="rec", tag="rec")
                    nc.vector.reciprocal_approx_fast(rec[:], deni[:])
                    recr = attnp.tile([1, 512], F32R, name="recr", tag="recr")
                    nc.vector.tensor_copy(recr[:], rec[:])
                    recrs.append(recr)
                return recrs

            def epilogue_ops(b, qc, recrs):
                """Broadcast + multiply; releases the ctx PSUM slots."""
                s = state[b]
                ctxs = s["ctx"].pop(qc)
                qlo = qc * 512
                ops = []
                for h in (0, 1):
                    def mk(h=h, ctx=ctxs[h], recr=recrs[h]):
                        def f():
                            bc = ps_st.tile([64, 512], F32, name="bc", tag="st")
                            nc.tensor.matmul(bc[:], onesr_sb[:], recr[:],
                                             start=True, stop=True)
                            bcs = attnp.tile([64, 512], F32, name="bcs",
                                             tag="bcs")
                            nc.vector.tensor_copy(bcs[:], bc[:])
                            nc.vector.tensor_mul(
                                s["ctxT"][h * 64:(h + 1) * 64, qlo: qlo + 512],
                                ctx[0:64, :], bcs[:])
                        return f
                    ops.append(mk(h))
                return [ops]

            # ---- prologue: batch 0 QKV + V-transpose, emitted densely ----
            alloc_batch(0)
            for tcn in range(NQ - 1):
                for ch in qkv_chunk_ops(0, tcn):
                    for f in ch:
                        f()
            for c in range(NK // 4 - 1):
                for ch in vtrans_chunk_ops(0, c):
                    for f in ch:
                        f()

            # ---- pipelined main loop ----
            prev = None  # (b, qc) awaiting epilogue
            for b in range(B):
                if b + 1 < B:
                    alloc_batch(b + 1)
                for qc in range(NQ):
                    filler = []
                    if prev is not None:
                        recrs = epilogue_pre(*prev)
                        filler += epilogue_ops(*prev, recrs)
                    if qc == 0:
                        # current batch's own tail chunk, deferred to here
                        filler += qkv_chunk_ops(b, 3)
                        filler += vtrans_chunk_ops(b, 3)
                    elif b + 1 < B:
                        filler += qkv_chunk_ops(b + 1, qc - 1)
                        filler += vtrans_chunk_ops(b + 1, qc - 1)
                    if b >= 1:
                        filler += proj_quarter_ops(b - 1, qc)
                    if b == B - 1 and qc >= 1:
                        filler += proj_quarter_ops(b, qc - 1)
                    kj_stream(b, qc, filler)
                    prev = (b, qc)
            recrs = epilogue_pre(*prev)
            for ch in epilogue_ops(*prev, recrs):
                for f in ch:
                    f()
            for ch in proj_quarter_ops(B - 1, 3):
                for f in ch:
                    f()

    nc.compile()
    return nc


def make_in_maps(x, Wq, Wk, Wv, Wo):
    xT = round_fp32r(np.ascontiguousarray(x.reshape(TT, D).T))
    tri = round_fp32r(np.triu(np.ones((128, 128), np.float32)))  # keep k<=q
    ident = round_fp32r(np.eye(128, dtype=np.float32))
    onesr = np.ones((1, 64), np.float32)
    ones32 = np.ones((128, 32), np.float32)
    in_maps = []
    for c in range(NCORES):
        dsl = slice(c * 128, (c + 1) * 128)
        wq = np.concatenate(
            [Wq[dsl, ct * 128:(ct + 1) * 128].T for ct in range(CT)], axis=1)
        wk = np.concatenate(
            [Wk[dsl, ct * 128:(ct + 1) * 128].T for ct in range(CT)], axis=1)
        wv = np.concatenate(
            [Wv[dsl, ct * 128:(ct + 1) * 128].T for ct in range(CT)], axis=1)
        wo = Wo[:, dsl].T
        in_maps.append({
            "xT": xT,
            "wq": round_fp32r(np.ascontiguousarray(wq)),
            "wk": round_fp32r(np.ascontiguousarray(wk)),
            "wv": round_fp32r(np.ascontiguousarray(wv)),
            "wo": round_fp32r(np.ascontiguousarray(wo)),
            "tri": tri, "ident": ident, "onesr": onesr, "ones32": ones32,
        })
    return in_maps


_NC_CACHE = None


def kernel_run(x, Wq, Wk, Wv, Wo, bo, trace=False, trace_cores=None):
    global _NC_CACHE
    if _NC_CACHE is None:
        _NC_CACHE = build_nc()
    nc = _NC_CACHE
    in_maps = make_in_maps(np.asarray(x), np.asarray(Wq), np.asarray(Wk),
                           np.asarray(Wv), np.asarray(Wo))
    res = run_bass_kernel_spmd(nc, in_maps, core_ids=list(range(NCORES)),
                               trace=trace, trace_cores=trace_cores)
    acc = res.results[0]["out"].astype(np.float64)
    for c in range(1, NCORES):
        acc += res.results[c]["out"]
    outv = (acc + np.asarray(bo, dtype=np.float64)).astype(np.float32)
    return outv.reshape(B, T, D), res


def kernel(x, Wq, Wk, Wv, Wo, bo):
    out, _ = kernel_run(x, Wq, Wk, Wv, Wo, bo)
    return out
